# revision 7
# baseline (speedup 1.0000x reference)
"""Trainium2 Bass kernel for the autoregressive GRU decode head.

Problem: context = mean over zones of encoded_features[:, -1]  -> (B, D)
then 12 autoregressive steps of a 2-layer GRU (H=256) + linear projection
to N=256 zones.  B=1024, data-parallel across 8 NeuronCores (128 batch each).

Device layout (per core, feature-major / "transposed" activations):
  actT (128p, 6 slots, 128) bf16 : [pred c0, pred c1, h0 c0, h0 c1, h1 c0, h1 c1]
       slot s, element [p, b] = act[b, c*128 + p]   (c = chunk of the 256-dim)
  All gate tensors (PSUM and SBUF) use layout [p, c*128 + b].
  Matmuls: out(gate_chunk, B) = lhsT.T @ rhs with lhsT = W^T tile (K=128, M=128),
  rhs = actT slot (K=128, B=128), accumulated over K chunks in PSUM.
"""

import sys

for _p in ("/opt/trn_rl_repo",):
    if _p not in sys.path:
        sys.path.insert(0, _p)

import numpy as np
import ml_dtypes

import concourse.bass as bass
import concourse.tile as tile
from concourse import mybir
from concourse.vector_clock import ScopedClock

BF16 = ml_dtypes.bfloat16

B, T, NZ, D = 1024, 8, 256, 256
H = 256
STEPS = 12
N_CORES = 8
PC = B // N_CORES  # 128 batch per core

F32 = mybir.dt.float32
BF = mybir.dt.bfloat16
AF = mybir.ActivationFunctionType
OP = mybir.AluOpType

# bias column layout in the (128, NBIAS) f32 bias tile
_BRZ0 = 0                      # 12 steps * 4 chunks
_BIN0 = _BRZ0 + STEPS * 4      # 12 steps * 2 chunks
_BHN0 = _BIN0 + STEPS * 2      # 2
_BRZ1 = _BHN0 + 2              # 4
_BIN1 = _BRZ1 + 4              # 2
_BHN1 = _BIN1 + 2              # 2
_BPT = _BHN1 + 2               # 2 (b_out for predT chunks)
NBIAS = _BPT + 2


def _install_tile_drain_patch():
    """walrus (CoreV3) rejects >1 sync wait on the tail drain; spill extras
    onto preceding sync nops."""
    if getattr(tile.TileContext, "_drain_patch_installed", False):
        return

    def _patched(self, tick_clock, wait_clock):
        nc = self.nc
        bb = nc.cur_bb.bb
        drain_bi = nc.sync.drain()
        drain_inst = drain_bi.ins
        wait_clock.add_sem_waits(
            drain_inst, ScopedClock({None: tick_clock.global_clock})
        )
        w = drain_inst.sync_info.on_wait if drain_inst.sync_info else None
        maxw = 1
        if w and len(w) > maxw:
            extra = list(w[maxw:])
            drain_inst.sync_info.on_wait = list(w[:maxw])
            idx = bb.instructions.index(drain_inst)
            nops = []
            for i in range(0, len(extra), maxw):
                nop_bi = nc.sync.nop()
                nop = nop_bi.ins
                si = nop.sync_info
                nop.sync_info = mybir.SyncInfo(
                    on_wait=extra[i : i + maxw],
                    on_update=(si.on_update if si else []),
                )
                bb.instructions.remove(nop)
                nops.append(nop)
            bb.instructions[idx:idx] = nops
        nc.all_engine_barrier()
        popped = nc._tile_sem_poison_stack.pop()
        assert popped is self._sem_poison
        nc.clear_and_free_semaphores(list(self.sems.allocated().values()))
        nc.all_engine_barrier()

    tile.TileContext._drain_and_barrier = _patched
    tile.TileContext._drain_patch_installed = True


def _split_waits(nc, maxw=1):
    """This walrus build rejects instructions carrying more than ~1 sem
    wait; spill extra waits onto same-engine nops placed just before."""
    for bb in nc.main_func.blocks:
        new_list = []
        for inst in bb.instructions:
            si = inst.sync_info
            w = list(si.on_wait) if si and si.on_wait else []
            if len(w) > maxw:
                keep = w[len(w) - maxw:]
                extra = w[: len(w) - maxw]
                si.on_wait = keep
                for i in range(0, len(extra), maxw):
                    nop = mybir.InstNoOp(
                        name=f"{inst.name}-sw{i}", ins=[], outs=[]
                    )
                    nop.engine = inst.engine
                    nop.sync_info = mybir.SyncInfo(
                        on_wait=extra[i : i + maxw], on_update=[]
                    )
                    nc.register_instruction(nop)
                    new_list.append(nop)
            new_list.append(inst)
        bb.instructions[:] = new_list


def build_kernel(debug_dump=()):
    """Build the per-core Bass graph (SPMD: same graph on all 8 cores).

    debug_dump: iterable of tags to dump as extra DRAM outputs, e.g.
    ("ctx", "s00", "h0f0", "n00").
    """
    _install_tile_drain_patch()
    nc = bass.Bass()
    dbg = {}
    for tag in debug_dump:
        dbg[tag] = nc.declare_dram_parameter(f"dbg_{tag}", [128, 512], F32,
                                             isOutput=True)

    enc = nc.declare_dram_parameter("enc", [PC, NZ, D], F32, isOutput=False)
    wrz0 = nc.declare_dram_parameter("wrz0", [4, 128, 512], BF, isOutput=False)
    win0 = nc.declare_dram_parameter("win0", [2, 128, 256], BF, isOutput=False)
    whn0 = nc.declare_dram_parameter("whn0", [2, 128, 256], BF, isOutput=False)
    wrz1 = nc.declare_dram_parameter("wrz1", [4, 128, 512], BF, isOutput=False)
    win1 = nc.declare_dram_parameter("win1", [2, 128, 256], BF, isOutput=False)
    whn1 = nc.declare_dram_parameter("whn1", [2, 128, 256], BF, isOutput=False)
    wout = nc.declare_dram_parameter("wout", [2, 128, 256], BF, isOutput=False)
    biases = nc.declare_dram_parameter("biases", [128, NBIAS], F32, isOutput=False)
    boutb = nc.declare_dram_parameter("boutb", [128, 256], F32, isOutput=False)
    out = nc.declare_dram_parameter("out", [PC, STEPS, NZ], F32, isOutput=True)

    with tile.TileContext(nc) as tc:
        with (
            tc.tile_pool(name="consts", bufs=1) as consts,
            tc.tile_pool(name="state", bufs=1) as state,
            tc.tile_pool(name="enc_pool", bufs=2) as enc_pool,
            tc.tile_pool(name="gates", bufs=2) as gates,
            tc.tile_pool(name="ostage", bufs=2) as ostage,
            tc.tile_pool(name="psum", bufs=1, space="PSUM") as psum,
        ):
            # ---- weight / bias loads (overlap with enc streaming) ----
            w_sb = {}
            for name, ap, kc, mdim in (
                ("wrz0", wrz0, 4, 512),
                ("win0", win0, 2, 256),
                ("whn0", whn0, 2, 256),
                ("wrz1", wrz1, 4, 512),
                ("win1", win1, 2, 256),
                ("whn1", whn1, 2, 256),
                ("wout", wout, 2, 256),
            ):
                t_ = consts.tile([128, kc, mdim], BF, tag=name)
                nc.sync.dma_start(t_[:], ap.rearrange("k p m -> p k m"))
                w_sb[name] = t_
            bias_sb = consts.tile([128, NBIAS], F32, tag="bias")
            nc.sync.dma_start(bias_sb[:], biases[:])
            boutb_sb = consts.tile([128, 256], F32, tag="boutb")
            nc.sync.dma_start(boutb_sb[:], boutb[:])

            identity = consts.tile([128, 128], F32, tag="ident")
            nc.gpsimd.memset(identity[:], 0.0)
            nc.gpsimd.affine_select(
                out=identity[:],
                in_=identity[:],
                compare_op=OP.not_equal,
                fill=1.0,
                base=0,
                pattern=[[-1, 128]],
                channel_multiplier=1,
            )

            # ---- phase 1: context = mean over zones of enc ----
            # enc (128b, 256z, 256d); stream in 4 chunks of 64 zones,
            # reduce over z on DVE (innermost axis after AP transpose).
            ZCH = 64
            NCH = NZ // ZCH
            partials = state.tile([128, NCH, D], F32, tag="partials")
            for i in range(NCH):
                e_sb = enc_pool.tile([128, ZCH * D], F32, tag="echunk")
                nc.sync.dma_start(e_sb[:], enc[:, i * ZCH : (i + 1) * ZCH, :])
                ev = e_sb[:].rearrange("p (z d) -> p d z", z=ZCH)
                nc.vector.tensor_reduce(
                    partials[:, i, :], ev, axis=mybir.AxisListType.X, op=OP.add
                )
            ctx = state.tile([128, D], F32, tag="ctx")
            nc.vector.tensor_tensor(
                partials[:, 0, :], partials[:, 0, :], partials[:, 1, :], OP.add
            )
            nc.vector.tensor_tensor(
                partials[:, 2, :], partials[:, 2, :], partials[:, 3, :], OP.add
            )
            nc.vector.tensor_tensor(
                ctx[:], partials[:, 0, :], partials[:, 2, :], OP.add
            )
            # scale by 1/NZ
            nc.scalar.mul(ctx[:], ctx[:], 1.0 / NZ)

            # ---- state tiles ----
            actT = state.tile([128, 6, 128], BF, tag="actT")
            h0f = state.tile([128, 256], F32, tag="h0f")
            h1f = state.tile([128, 256], F32, tag="h1f")

            # transpose ctx (b-part) -> ctxT (d-part) chunks; init h0/h1
            for c in range(2):
                ctps = psum.tile([128, 128], F32, tag="pred")
                nc.tensor.transpose(ctps[:], ctx[:, c * 128 : (c + 1) * 128], identity[:])
                nc.scalar.activation(actT[:, 2 + c, :], ctps[:], AF.Copy)
                nc.vector.tensor_copy(h0f[:, c * 128 : (c + 1) * 128], ctps[:])
            nc.scalar.activation(actT[:, 4, :], actT[:, 2, :], AF.Copy)
            nc.scalar.activation(actT[:, 5, :], actT[:, 3, :], AF.Copy)
            nc.vector.tensor_copy(h1f[:], h0f[:])
            nc.vector.memset(actT[:, 0:2, :], 0.0)

            # ---- phase 2: 12 decode steps ----
            def gru_layer(layer, t):
                """Compute h' for one GRU layer; returns nothing (updates
                actT slots + h{l}f in place)."""
                if layer == 0:
                    w_rz, w_in, w_hn = w_sb["wrz0"], w_sb["win0"], w_sb["whn0"]
                    # (weight K-chunk index, actT slot) in issue order
                    rz_ks = ((0, 0), (1, 1), (2, 2), (3, 3))  # pred, h0
                    in_slots = (0, 1)         # K = pred
                    hn_slots = (2, 3)         # K = h0 (own hidden)
                    h_slots = (2, 3)
                    hf = h0f
                    brz_col = _BRZ0 + t * 4
                    bin_col = _BIN0 + t * 2
                    bhn_col = _BHN0
                else:
                    w_rz, w_in, w_hn = w_sb["wrz1"], w_sb["win1"], w_sb["whn1"]
                    # wrz1 K-chunk rows are [h0c0, h0c1, h1c0, h1c1]; issue the
                    # h1 (old) chunks first, the h0' chunks when ready
                    rz_ks = ((2, 4), (3, 5), (0, 2), (1, 3))
                    in_slots = (2, 3)         # K = h0'
                    hn_slots = (4, 5)
                    h_slots = (4, 5)
                    hf = h1f
                    brz_col = _BRZ1
                    bin_col = _BIN1
                    bhn_col = _BHN1

                g_rz = psum.tile([128, 512], F32, tag="rz")
                g_hn = psum.tile([128, 256], F32, tag="nhn")
                g_in = psum.tile([128, 256], F32, tag="nin")
                # K-chunk weight-row index for each rhs slot: the weight
                # arrays were built in the same order as rz_slots
                for m in range(4):
                    ms = slice(m * 128, (m + 1) * 128)
                    for j, (ki, slot) in enumerate(rz_ks):
                        nc.tensor.matmul(
                            g_rz[:, ms],
                            w_rz[:, ki, ms],
                            actT[:, slot, :],
                            start=(j == 0),
                            stop=(j == len(rz_ks) - 1),
                        )
                for m in range(2):
                    ms = slice(m * 128, (m + 1) * 128)
                    for ki, slot in enumerate(hn_slots):
                        nc.tensor.matmul(
                            g_hn[:, ms],
                            w_hn[:, ki, ms],
                            actT[:, slot, :],
                            start=(ki == 0),
                            stop=(ki == 1),
                        )
                    for ki, slot in enumerate(in_slots):
                        nc.tensor.matmul(
                            g_in[:, ms],
                            w_in[:, ki, ms],
                            actT[:, slot, :],
                            start=(ki == 0),
                            stop=(ki == 1),
                        )
                # gates
                s_ = gates.tile([128, 512], F32, tag="S")
                for c in range(4):
                    cs = slice(c * 128, (c + 1) * 128)
                    nc.scalar.activation(
                        s_[:, cs], g_rz[:, cs], AF.Sigmoid,
                        bias=bias_sb[:, brz_col + c : brz_col + c + 1],
                    )
                t_ = gates.tile([128, 256], F32, tag="tt")
                v_ = gates.tile([128, 256], F32, tag="vv")
                for c in range(2):
                    cs = slice(c * 128, (c + 1) * 128)
                    # t = (h_n + bhn) * r
                    nc.vector.scalar_tensor_tensor(
                        t_[:, cs], g_hn[:, cs],
                        bias_sb[:, bhn_col + c : bhn_col + c + 1],
                        s_[:, cs], op0=OP.add, op1=OP.mult,
                    )
                    # v = (i_n + bin) + t
                    nc.vector.scalar_tensor_tensor(
                        v_[:, cs], g_in[:, cs],
                        bias_sb[:, bin_col + c : bin_col + c + 1],
                        t_[:, cs], op0=OP.add, op1=OP.add,
                    )
                n_ = gates.tile([128, 256], F32, tag="nn")
                nc.scalar.activation(n_[:], v_[:], AF.Tanh)
                d_ = gates.tile([128, 256], F32, tag="dd")
                e_ = gates.tile([128, 256], F32, tag="ee")
                nc.vector.tensor_tensor(d_[:], hf[:], n_[:], OP.subtract)
                nc.vector.tensor_tensor(e_[:], s_[:, 256:512], d_[:], OP.mult)
                nc.vector.tensor_tensor(hf[:], n_[:], e_[:], OP.add)
                # bf16 copy for next matmuls
                nc.scalar.activation(
                    actT[:, h_slots[0] : h_slots[0] + 2, :], hf[:], AF.Copy
                )
                key = f"{layer}{t}"
                if f"s{key}" in dbg:
                    nc.sync.dma_start(dbg[f"s{key}"][:], s_[:])
                if f"grz{key}" in dbg:
                    g_ = gates.tile([128, 512], F32, tag="gdump")
                    nc.vector.tensor_copy(g_[:], g_rz[:])
                    nc.sync.dma_start(dbg[f"grz{key}"][:], g_[:])
                if f"v{key}" in dbg:
                    nc.sync.dma_start(dbg[f"v{key}"][:, 0:256], v_[:])
                if f"n{key}" in dbg:
                    nc.sync.dma_start(dbg[f"n{key}"][:, 0:256], n_[:])
                if f"h{key}" in dbg:
                    nc.sync.dma_start(dbg[f"h{key}"][:, 0:256], hf[:])

            for t in range(STEPS):
                gru_layer(0, t)
                gru_layer(1, t)
                # projection: predT (for recurrence) + predB (for output)
                g_pt = psum.tile([128, 256], F32, tag="pred")
                for m in range(2):
                    ms = slice(m * 128, (m + 1) * 128)
                    for ki, slot in enumerate((4, 5)):
                        nc.tensor.matmul(
                            g_pt[:, ms],
                            w_sb["wout"][:, ki, ms],
                            actT[:, slot, :],
                            start=(ki == 0),
                            stop=(ki == 1),
                        )
                g_pb = psum.tile([128, 256], F32, tag="predB")
                for ki, slot in enumerate((4, 5)):
                    nc.tensor.matmul(
                        g_pb[:],
                        actT[:, slot, :],
                        w_sb["wout"][:, ki, :],
                        start=(ki == 0),
                        stop=(ki == 1),
                    )
                for c in range(2):
                    cs = slice(c * 128, (c + 1) * 128)
                    nc.scalar.activation(
                        actT[:, c, :], g_pt[:, cs], AF.Identity,
                        bias=bias_sb[:, _BPT + c : _BPT + c + 1],
                    )
                o_ = ostage.tile([128, 256], F32, tag="ost")
                nc.vector.tensor_tensor(o_[:], g_pb[:], boutb_sb[:], OP.add)
                nc.sync.dma_start(out[:, t, :], o_[:])

    _split_waits(nc)
    return nc


def _prep_inputs(encoded_features, step_emb, W_ih0, W_hh0, b_ih0, b_hh0,
                 W_ih1, W_hh1, b_ih1, b_hh1, W_out, b_out):
    """Host-side: slice/shard the big input, transpose + cast weights,
    fold step-embedding matmul + biases into per-step bias vectors."""
    f4 = np.float32
    enc_last = np.ascontiguousarray(np.asarray(encoded_features)[:, -1], dtype=f4)

    W_ih0 = np.asarray(W_ih0, f4)
    W_hh0 = np.asarray(W_hh0, f4)
    W_ih1 = np.asarray(W_ih1, f4)
    W_hh1 = np.asarray(W_hh1, f4)
    W_out = np.asarray(W_out, f4)
    step_emb = np.asarray(step_emb, f4)
    b_ih0 = np.asarray(b_ih0, f4)
    b_hh0 = np.asarray(b_hh0, f4)
    b_ih1 = np.asarray(b_ih1, f4)
    b_hh1 = np.asarray(b_hh1, f4)
    b_out = np.asarray(b_out, f4)

    W_emb = W_ih0[:, :D]          # (768, 256)
    W_pred = W_ih0[:, D:]         # (768, 256)

    # gi_emb[t] = W_emb @ step_emb[t] + b_ih0  -> (12, 768)
    gi_emb = step_emb[:STEPS] @ W_emb.T + b_ih0[None, :]

    def kstack(*mats_cols):
        """Stack K-chunk lhsT tiles: each entry (mat, col_slice) contributes
        its .T rows split into 128-row chunks."""
        chunks = []
        for mat, cols in mats_cols:
            mt = np.ascontiguousarray(mat.T[:, cols])  # (K, M)
            for k in range(0, mt.shape[0], 128):
                chunks.append(mt[k : k + 128])
        return np.stack(chunks).astype(BF16)  # (nk, 128, M)

    rz = slice(0, 512)
    ng = slice(512, 768)
    wrz0 = kstack((W_pred, rz), (W_hh0, rz))          # K: pred0,pred1,h0c0,h0c1
    win0 = kstack((W_pred, ng))
    whn0 = kstack((W_hh0, ng))
    wrz1 = kstack((W_ih1, rz), (W_hh1, rz))           # K: h0c0,h0c1,h1c0,h1c1
    win1 = kstack((W_ih1, ng))
    whn1 = kstack((W_hh1, ng))
    wout = np.stack([np.ascontiguousarray(W_out.T)[k : k + 128] for k in (0, 128)]
                    ).astype(BF16)                    # (2, 128, 256)

    biases = np.zeros((128, NBIAS), f4)
    for t in range(STEPS):
        brz = gi_emb[t, :512] + b_hh0[:512]
        for c in range(4):
            biases[:, _BRZ0 + t * 4 + c] = brz[c * 128 : (c + 1) * 128]
        bin_ = gi_emb[t, 512:]
        for c in range(2):
            biases[:, _BIN0 + t * 2 + c] = bin_[c * 128 : (c + 1) * 128]
    for c in range(2):
        biases[:, _BHN0 + c] = b_hh0[512 + c * 128 : 512 + (c + 1) * 128]
        biases[:, _BIN1 + c] = b_ih1[512 + c * 128 : 512 + (c + 1) * 128]
        biases[:, _BHN1 + c] = b_hh1[512 + c * 128 : 512 + (c + 1) * 128]
        biases[:, _BPT + c] = b_out[c * 128 : (c + 1) * 128]
    brz1 = b_ih1[:512] + b_hh1[:512]
    for c in range(4):
        biases[:, _BRZ1 + c] = brz1[c * 128 : (c + 1) * 128]
    boutb = np.broadcast_to(b_out[None, :], (128, 256)).copy()

    shared = dict(wrz0=wrz0, win0=win0, whn0=whn0, wrz1=wrz1, win1=win1,
                  whn1=whn1, wout=wout, biases=biases, boutb=boutb)
    in_maps = []
    for i in range(N_CORES):
        m = dict(shared)
        m["enc"] = enc_last[i * PC : (i + 1) * PC]
        in_maps.append(m)
    return in_maps


_CACHE = {}


def _run(in_maps, trace=False):
    from concourse.bass_utils import run_bass_kernel_spmd

    if "nc" not in _CACHE:
        _CACHE["nc"] = build_kernel()
    nc = _CACHE["nc"]
    res = run_bass_kernel_spmd(
        nc, in_maps, core_ids=list(range(N_CORES)), trace=trace
    )
    preds = np.concatenate([res.results[i]["out"] for i in range(N_CORES)], axis=0)
    return preds, res


def kernel(encoded_features, step_emb, W_ih0, W_hh0, b_ih0, b_hh0,
           W_ih1, W_hh1, b_ih1, b_hh1, W_out, b_out, num_steps):
    assert int(num_steps) == STEPS
    in_maps = _prep_inputs(encoded_features, step_emb, W_ih0, W_hh0, b_ih0,
                           b_hh0, W_ih1, W_hh1, b_ih1, b_hh1, W_out, b_out)
    preds, _ = _run(in_maps, trace=False)
    return preds


# revision 9
# speedup vs baseline: 1.1029x; 1.1029x over previous
"""Trainium2 Bass kernel for the autoregressive GRU decode head.

Problem: context = mean over zones of encoded_features[:, -1]  -> (B, D)
then 12 autoregressive steps of a 2-layer GRU (H=256) + linear projection
to N=256 zones.  B=1024, data-parallel across 8 NeuronCores (128 batch each).

Device layout (per core, feature-major / "transposed" activations):
  actT (128p, 6 slots, 128) bf16 : [pred c0, pred c1, h0 c0, h0 c1, h1 c0, h1 c1]
       slot s, element [p, b] = act[b, c*128 + p]   (c = chunk of the 256-dim)
  All gate tensors (PSUM and SBUF) use layout [p, c*128 + b].
  Matmuls: out(gate_chunk, B) = lhsT.T @ rhs with lhsT = W^T tile (K=128, M=128),
  rhs = actT slot (K=128, B=128), accumulated over K chunks in PSUM.
"""

import sys

for _p in ("/opt/trn_rl_repo",):
    if _p not in sys.path:
        sys.path.insert(0, _p)

import numpy as np
import ml_dtypes

import concourse.bass as bass
import concourse.tile as tile
from concourse import mybir
from concourse.vector_clock import ScopedClock

BF16 = ml_dtypes.bfloat16

B, T, NZ, D = 1024, 8, 256, 256
H = 256
STEPS = 12
N_CORES = 8
PC = B // N_CORES  # 128 batch per core

F32 = mybir.dt.float32
BF = mybir.dt.bfloat16
AF = mybir.ActivationFunctionType
OP = mybir.AluOpType

# bias column layout in the (128, NBIAS) f32 bias tile
_BRZ0 = 0                      # 12 steps * 4 chunks
_BIN0 = _BRZ0 + STEPS * 4      # 12 steps * 2 chunks
_BHN0 = _BIN0 + STEPS * 2      # 2
_BRZ1 = _BHN0 + 2              # 4
_BIN1 = _BRZ1 + 4              # 2
_BHN1 = _BIN1 + 2              # 2
_BPT = _BHN1 + 2               # 2 (b_out for predT chunks)
NBIAS = _BPT + 2


def _install_tile_drain_patch():
    """walrus (CoreV3) rejects >1 sync wait on the tail drain; spill extras
    onto preceding sync nops."""
    if getattr(tile.TileContext, "_drain_patch_installed", False):
        return

    def _patched(self, tick_clock, wait_clock):
        nc = self.nc
        bb = nc.cur_bb.bb
        drain_bi = nc.sync.drain()
        drain_inst = drain_bi.ins
        wait_clock.add_sem_waits(
            drain_inst, ScopedClock({None: tick_clock.global_clock})
        )
        w = drain_inst.sync_info.on_wait if drain_inst.sync_info else None
        maxw = 1
        if w and len(w) > maxw:
            extra = list(w[maxw:])
            drain_inst.sync_info.on_wait = list(w[:maxw])
            idx = bb.instructions.index(drain_inst)
            nops = []
            for i in range(0, len(extra), maxw):
                nop_bi = nc.sync.nop()
                nop = nop_bi.ins
                si = nop.sync_info
                nop.sync_info = mybir.SyncInfo(
                    on_wait=extra[i : i + maxw],
                    on_update=(si.on_update if si else []),
                )
                bb.instructions.remove(nop)
                nops.append(nop)
            bb.instructions[idx:idx] = nops
        nc.all_engine_barrier()
        popped = nc._tile_sem_poison_stack.pop()
        assert popped is self._sem_poison
        nc.clear_and_free_semaphores(list(self.sems.allocated().values()))
        nc.all_engine_barrier()

    tile.TileContext._drain_and_barrier = _patched
    tile.TileContext._drain_patch_installed = True


def _split_waits(nc, maxw=1):
    """This walrus build rejects instructions carrying more than ~1 sem
    wait; spill extra waits onto same-engine nops placed just before."""
    for bb in nc.main_func.blocks:
        new_list = []
        for inst in bb.instructions:
            si = inst.sync_info
            w = list(si.on_wait) if si and si.on_wait else []
            if len(w) > maxw:
                keep = w[len(w) - maxw:]
                extra = w[: len(w) - maxw]
                si.on_wait = keep
                for i in range(0, len(extra), maxw):
                    nop = mybir.InstNoOp(
                        name=f"{inst.name}-sw{i}", ins=[], outs=[]
                    )
                    nop.engine = inst.engine
                    nop.sync_info = mybir.SyncInfo(
                        on_wait=extra[i : i + maxw], on_update=[]
                    )
                    nc.register_instruction(nop)
                    new_list.append(nop)
            new_list.append(inst)
        bb.instructions[:] = new_list


def build_kernel(debug_dump=()):
    """Build the per-core Bass graph (SPMD: same graph on all 8 cores).

    debug_dump: iterable of tags to dump as extra DRAM outputs, e.g.
    ("ctx", "s00", "h0f0", "n00").
    """
    _install_tile_drain_patch()
    nc = bass.Bass()
    dbg = {}
    for tag in debug_dump:
        dbg[tag] = nc.declare_dram_parameter(f"dbg_{tag}", [128, 512], F32,
                                             isOutput=True)

    enc = nc.declare_dram_parameter("enc", [PC, NZ, D], F32, isOutput=False)
    wrz0 = nc.declare_dram_parameter("wrz0", [4, 128, 512], BF, isOutput=False)
    win0 = nc.declare_dram_parameter("win0", [2, 128, 256], BF, isOutput=False)
    whn0 = nc.declare_dram_parameter("whn0", [2, 128, 256], BF, isOutput=False)
    wrz1 = nc.declare_dram_parameter("wrz1", [4, 128, 512], BF, isOutput=False)
    win1 = nc.declare_dram_parameter("win1", [2, 128, 256], BF, isOutput=False)
    whn1 = nc.declare_dram_parameter("whn1", [2, 128, 256], BF, isOutput=False)
    wout = nc.declare_dram_parameter("wout", [2, 128, 256], BF, isOutput=False)
    biases = nc.declare_dram_parameter("biases", [128, NBIAS], F32, isOutput=False)
    boutb = nc.declare_dram_parameter("boutb", [128, 256], F32, isOutput=False)
    out = nc.declare_dram_parameter("out", [PC, STEPS, NZ], F32, isOutput=True)

    with tile.TileContext(nc) as tc:
        with (
            tc.tile_pool(name="consts", bufs=1) as consts,
            tc.tile_pool(name="state", bufs=1) as state,
            tc.tile_pool(name="enc_pool", bufs=2) as enc_pool,
            tc.tile_pool(name="gates", bufs=2) as gates,
            tc.tile_pool(name="ostage", bufs=2) as ostage,
            tc.tile_pool(name="psum", bufs=1, space="PSUM") as psum,
        ):
            # ---- weight / bias loads (overlap with enc streaming) ----
            w_sb = {}
            for name, ap, kc, mdim in (
                ("wrz0", wrz0, 4, 512),
                ("win0", win0, 2, 256),
                ("whn0", whn0, 2, 256),
                ("wrz1", wrz1, 4, 512),
                ("win1", win1, 2, 256),
                ("whn1", whn1, 2, 256),
                ("wout", wout, 2, 256),
            ):
                t_ = consts.tile([128, kc, mdim], BF, tag=name)
                nc.sync.dma_start(t_[:], ap.rearrange("k p m -> p k m"))
                w_sb[name] = t_
            bias_sb = consts.tile([128, NBIAS], F32, tag="bias")
            nc.sync.dma_start(bias_sb[:], biases[:])
            boutb_sb = consts.tile([128, 256], F32, tag="boutb")
            nc.sync.dma_start(boutb_sb[:], boutb[:])

            identity = consts.tile([128, 128], F32, tag="ident")
            nc.gpsimd.memset(identity[:], 0.0)
            nc.gpsimd.affine_select(
                out=identity[:],
                in_=identity[:],
                compare_op=OP.not_equal,
                fill=1.0,
                base=0,
                pattern=[[-1, 128]],
                channel_multiplier=1,
            )

            # ---- phase 1: context = mean over zones of enc ----
            # enc (128b, 256z, 256d); stream in 4 chunks of 64 zones,
            # reduce over z on DVE (innermost axis after AP transpose).
            ZCH = 64
            NCH = NZ // ZCH
            partials = state.tile([128, NCH, D], F32, tag="partials")
            for i in range(NCH):
                e_sb = enc_pool.tile([128, ZCH * D], F32, tag="echunk")
                nc.sync.dma_start(e_sb[:], enc[:, i * ZCH : (i + 1) * ZCH, :])
                # contiguous pairwise-halving tree: sum over the 64 zones
                w = ZCH * D
                while w > 2 * D:
                    h = w // 2
                    nc.vector.tensor_tensor(
                        e_sb[:, 0:h], e_sb[:, 0:h], e_sb[:, h:w], OP.add
                    )
                    w = h
                nc.vector.tensor_tensor(
                    partials[:, i, :], e_sb[:, 0:D], e_sb[:, D : 2 * D], OP.add
                )
            ctx = state.tile([128, D], F32, tag="ctx")
            nc.vector.tensor_tensor(
                partials[:, 0, :], partials[:, 0, :], partials[:, 1, :], OP.add
            )
            nc.vector.tensor_tensor(
                partials[:, 2, :], partials[:, 2, :], partials[:, 3, :], OP.add
            )
            nc.vector.tensor_tensor(
                ctx[:], partials[:, 0, :], partials[:, 2, :], OP.add
            )
            # scale by 1/NZ
            nc.scalar.mul(ctx[:], ctx[:], 1.0 / NZ)

            # ---- state tiles ----
            actT = state.tile([128, 6, 128], BF, tag="actT")
            h0f = state.tile([128, 256], F32, tag="h0f")
            h1f = state.tile([128, 256], F32, tag="h1f")

            # transpose ctx (b-part) -> ctxT (d-part) chunks; init h0/h1
            for c in range(2):
                ctps = psum.tile([128, 128], F32, tag="pred")
                nc.tensor.transpose(ctps[:], ctx[:, c * 128 : (c + 1) * 128], identity[:])
                nc.scalar.activation(actT[:, 2 + c, :], ctps[:], AF.Copy)
                nc.vector.tensor_copy(h0f[:, c * 128 : (c + 1) * 128], ctps[:])
            nc.scalar.activation(actT[:, 4, :], actT[:, 2, :], AF.Copy)
            nc.scalar.activation(actT[:, 5, :], actT[:, 3, :], AF.Copy)
            nc.vector.tensor_copy(h1f[:], h0f[:])
            nc.vector.memset(actT[:, 0:2, :], 0.0)

            # ---- phase 2: 12 decode steps ----
            def gru_layer(layer, t):
                """Compute h' for one GRU layer; returns nothing (updates
                actT slots + h{l}f in place)."""
                if layer == 0:
                    w_rz, w_in, w_hn = w_sb["wrz0"], w_sb["win0"], w_sb["whn0"]
                    # (weight K-chunk index, actT slot) in issue order
                    rz_ks = ((0, 0), (1, 1), (2, 2), (3, 3))  # pred, h0
                    in_slots = (0, 1)         # K = pred
                    hn_slots = (2, 3)         # K = h0 (own hidden)
                    h_slots = (2, 3)
                    hf = h0f
                    brz_col = _BRZ0 + t * 4
                    bin_col = _BIN0 + t * 2
                    bhn_col = _BHN0
                else:
                    w_rz, w_in, w_hn = w_sb["wrz1"], w_sb["win1"], w_sb["whn1"]
                    # wrz1 K-chunk rows are [h0c0, h0c1, h1c0, h1c1]; issue the
                    # h1 (old) chunks first, the h0' chunks when ready
                    rz_ks = ((2, 4), (3, 5), (0, 2), (1, 3))
                    in_slots = (2, 3)         # K = h0'
                    hn_slots = (4, 5)
                    h_slots = (4, 5)
                    hf = h1f
                    brz_col = _BRZ1
                    bin_col = _BIN1
                    bhn_col = _BHN1

                g_rz = psum.tile([128, 512], F32, tag="rz", bufs=2)
                g_hn = psum.tile([128, 256], F32, tag="nhn", bufs=2)
                g_in = psum.tile([128, 256], F32, tag="nin", bufs=2)
                # K-chunk weight-row index for each rhs slot: the weight
                # arrays were built in the same order as rz_slots
                for m in range(4):
                    ms = slice(m * 128, (m + 1) * 128)
                    for j, (ki, slot) in enumerate(rz_ks):
                        nc.tensor.matmul(
                            g_rz[:, ms],
                            w_rz[:, ki, ms],
                            actT[:, slot, :],
                            start=(j == 0),
                            stop=(j == len(rz_ks) - 1),
                        )
                for m in range(2):
                    ms = slice(m * 128, (m + 1) * 128)
                    for ki, slot in enumerate(hn_slots):
                        nc.tensor.matmul(
                            g_hn[:, ms],
                            w_hn[:, ki, ms],
                            actT[:, slot, :],
                            start=(ki == 0),
                            stop=(ki == 1),
                        )
                    for ki, slot in enumerate(in_slots):
                        nc.tensor.matmul(
                            g_in[:, ms],
                            w_in[:, ki, ms],
                            actT[:, slot, :],
                            start=(ki == 0),
                            stop=(ki == 1),
                        )
                # gates
                s_ = gates.tile([128, 512], F32, tag="S")
                for c in range(4):
                    cs = slice(c * 128, (c + 1) * 128)
                    nc.scalar.activation(
                        s_[:, cs], g_rz[:, cs], AF.Sigmoid,
                        bias=bias_sb[:, brz_col + c : brz_col + c + 1],
                    )
                t_ = gates.tile([128, 256], F32, tag="tt")
                v_ = gates.tile([128, 256], F32, tag="vv")
                for c in range(2):
                    cs = slice(c * 128, (c + 1) * 128)
                    # t = (h_n + bhn) * r
                    nc.vector.scalar_tensor_tensor(
                        t_[:, cs], g_hn[:, cs],
                        bias_sb[:, bhn_col + c : bhn_col + c + 1],
                        s_[:, cs], op0=OP.add, op1=OP.mult,
                    )
                    # v = (i_n + bin) + t
                    nc.vector.scalar_tensor_tensor(
                        v_[:, cs], g_in[:, cs],
                        bias_sb[:, bin_col + c : bin_col + c + 1],
                        t_[:, cs], op0=OP.add, op1=OP.add,
                    )
                n_ = gates.tile([128, 256], F32, tag="nn")
                nc.scalar.activation(n_[:], v_[:], AF.Tanh)
                d_ = gates.tile([128, 256], F32, tag="dd")
                e_ = gates.tile([128, 256], F32, tag="ee")
                nc.vector.tensor_tensor(d_[:], hf[:], n_[:], OP.subtract)
                nc.vector.tensor_tensor(e_[:], s_[:, 256:512], d_[:], OP.mult)
                nc.vector.tensor_tensor(hf[:], n_[:], e_[:], OP.add)
                # bf16 copy for next matmuls
                nc.scalar.activation(
                    actT[:, h_slots[0] : h_slots[0] + 2, :], hf[:], AF.Copy
                )
                key = f"{layer}{t}"
                if f"s{key}" in dbg:
                    nc.sync.dma_start(dbg[f"s{key}"][:], s_[:])
                if f"grz{key}" in dbg:
                    g_ = gates.tile([128, 512], F32, tag="gdump")
                    nc.vector.tensor_copy(g_[:], g_rz[:])
                    nc.sync.dma_start(dbg[f"grz{key}"][:], g_[:])
                if f"v{key}" in dbg:
                    nc.sync.dma_start(dbg[f"v{key}"][:, 0:256], v_[:])
                if f"n{key}" in dbg:
                    nc.sync.dma_start(dbg[f"n{key}"][:, 0:256], n_[:])
                if f"h{key}" in dbg:
                    nc.sync.dma_start(dbg[f"h{key}"][:, 0:256], hf[:])

            for t in range(STEPS):
                gru_layer(0, t)
                gru_layer(1, t)
                # projection: predT (for recurrence) + predB (for output)
                g_pt = psum.tile([128, 256], F32, tag="pred")
                for m in range(2):
                    ms = slice(m * 128, (m + 1) * 128)
                    for ki, slot in enumerate((4, 5)):
                        nc.tensor.matmul(
                            g_pt[:, ms],
                            w_sb["wout"][:, ki, ms],
                            actT[:, slot, :],
                            start=(ki == 0),
                            stop=(ki == 1),
                        )
                g_pb = psum.tile([128, 256], F32, tag="predB")
                for ki, slot in enumerate((4, 5)):
                    nc.tensor.matmul(
                        g_pb[:],
                        actT[:, slot, :],
                        w_sb["wout"][:, ki, :],
                        start=(ki == 0),
                        stop=(ki == 1),
                    )
                for c in range(2):
                    cs = slice(c * 128, (c + 1) * 128)
                    nc.scalar.activation(
                        actT[:, c, :], g_pt[:, cs], AF.Identity,
                        bias=bias_sb[:, _BPT + c : _BPT + c + 1],
                    )
                o_ = ostage.tile([128, 256], F32, tag="ost")
                nc.vector.tensor_tensor(o_[:], g_pb[:], boutb_sb[:], OP.add)
                nc.sync.dma_start(out[:, t, :], o_[:])

    _split_waits(nc)
    return nc


def _prep_inputs(encoded_features, step_emb, W_ih0, W_hh0, b_ih0, b_hh0,
                 W_ih1, W_hh1, b_ih1, b_hh1, W_out, b_out):
    """Host-side: slice/shard the big input, transpose + cast weights,
    fold step-embedding matmul + biases into per-step bias vectors."""
    f4 = np.float32
    enc_last = np.ascontiguousarray(np.asarray(encoded_features)[:, -1], dtype=f4)

    W_ih0 = np.asarray(W_ih0, f4)
    W_hh0 = np.asarray(W_hh0, f4)
    W_ih1 = np.asarray(W_ih1, f4)
    W_hh1 = np.asarray(W_hh1, f4)
    W_out = np.asarray(W_out, f4)
    step_emb = np.asarray(step_emb, f4)
    b_ih0 = np.asarray(b_ih0, f4)
    b_hh0 = np.asarray(b_hh0, f4)
    b_ih1 = np.asarray(b_ih1, f4)
    b_hh1 = np.asarray(b_hh1, f4)
    b_out = np.asarray(b_out, f4)

    W_emb = W_ih0[:, :D]          # (768, 256)
    W_pred = W_ih0[:, D:]         # (768, 256)

    # gi_emb[t] = W_emb @ step_emb[t] + b_ih0  -> (12, 768)
    gi_emb = step_emb[:STEPS] @ W_emb.T + b_ih0[None, :]

    def kstack(*mats_cols):
        """Stack K-chunk lhsT tiles: each entry (mat, col_slice) contributes
        its .T rows split into 128-row chunks."""
        chunks = []
        for mat, cols in mats_cols:
            mt = np.ascontiguousarray(mat.T[:, cols])  # (K, M)
            for k in range(0, mt.shape[0], 128):
                chunks.append(mt[k : k + 128])
        return np.stack(chunks).astype(BF16)  # (nk, 128, M)

    rz = slice(0, 512)
    ng = slice(512, 768)
    wrz0 = kstack((W_pred, rz), (W_hh0, rz))          # K: pred0,pred1,h0c0,h0c1
    win0 = kstack((W_pred, ng))
    whn0 = kstack((W_hh0, ng))
    wrz1 = kstack((W_ih1, rz), (W_hh1, rz))           # K: h0c0,h0c1,h1c0,h1c1
    win1 = kstack((W_ih1, ng))
    whn1 = kstack((W_hh1, ng))
    wout = np.stack([np.ascontiguousarray(W_out.T)[k : k + 128] for k in (0, 128)]
                    ).astype(BF16)                    # (2, 128, 256)

    biases = np.zeros((128, NBIAS), f4)
    for t in range(STEPS):
        brz = gi_emb[t, :512] + b_hh0[:512]
        for c in range(4):
            biases[:, _BRZ0 + t * 4 + c] = brz[c * 128 : (c + 1) * 128]
        bin_ = gi_emb[t, 512:]
        for c in range(2):
            biases[:, _BIN0 + t * 2 + c] = bin_[c * 128 : (c + 1) * 128]
    for c in range(2):
        biases[:, _BHN0 + c] = b_hh0[512 + c * 128 : 512 + (c + 1) * 128]
        biases[:, _BIN1 + c] = b_ih1[512 + c * 128 : 512 + (c + 1) * 128]
        biases[:, _BHN1 + c] = b_hh1[512 + c * 128 : 512 + (c + 1) * 128]
        biases[:, _BPT + c] = b_out[c * 128 : (c + 1) * 128]
    brz1 = b_ih1[:512] + b_hh1[:512]
    for c in range(4):
        biases[:, _BRZ1 + c] = brz1[c * 128 : (c + 1) * 128]
    boutb = np.broadcast_to(b_out[None, :], (128, 256)).copy()

    shared = dict(wrz0=wrz0, win0=win0, whn0=whn0, wrz1=wrz1, win1=win1,
                  whn1=whn1, wout=wout, biases=biases, boutb=boutb)
    in_maps = []
    for i in range(N_CORES):
        m = dict(shared)
        m["enc"] = enc_last[i * PC : (i + 1) * PC]
        in_maps.append(m)
    return in_maps


_CACHE = {}


def _run(in_maps, trace=False):
    from concourse.bass_utils import run_bass_kernel_spmd

    if "nc" not in _CACHE:
        _CACHE["nc"] = build_kernel()
    nc = _CACHE["nc"]
    res = run_bass_kernel_spmd(
        nc, in_maps, core_ids=list(range(N_CORES)), trace=trace
    )
    preds = np.concatenate([res.results[i]["out"] for i in range(N_CORES)], axis=0)
    return preds, res


def kernel(encoded_features, step_emb, W_ih0, W_hh0, b_ih0, b_hh0,
           W_ih1, W_hh1, b_ih1, b_hh1, W_out, b_out, num_steps):
    assert int(num_steps) == STEPS
    in_maps = _prep_inputs(encoded_features, step_emb, W_ih0, W_hh0, b_ih0,
                           b_hh0, W_ih1, W_hh1, b_ih1, b_hh1, W_out, b_out)
    preds, _ = _run(in_maps, trace=False)
    return preds


# revision 12
# speedup vs baseline: 1.5691x; 1.4227x over previous
"""Trainium2 Bass kernel for the autoregressive GRU decode head.

Problem: context = mean over zones of encoded_features[:, -1]  -> (B, D)
then 12 autoregressive steps of a 2-layer GRU (H=256) + linear projection
to N=256 zones.  B=1024, data-parallel across 8 NeuronCores (128 batch each).

Device layout (per core, feature-major / "transposed" activations):
  actT (128p, 6 slots, 128) bf16 : [pred c0, pred c1, h0 c0, h0 c1, h1 c0, h1 c1]
       slot s holds act[b, c*128 + p] at [p, b]    (c = chunk of the 256-dim)
  All gate tensors (PSUM and SBUF) use layout [p, c*128 + b].
  Matmuls: out(gate_chunk, B) = lhsT.T @ rhs with lhsT = W^T tile (K=128, M=128),
  rhs = actT slot (K=128, B=128), accumulated over K chunks in PSUM.
The encoded_features slice is streamed as bf16 (host-converted); the zone mean
is a pairwise TT-add tree on DVE (bf16 2x for the big levels, f32 tail).
"""

import sys

for _p in ("/opt/trn_rl_repo",):
    if _p not in sys.path:
        sys.path.insert(0, _p)

import numpy as np
import ml_dtypes

import concourse.bass as bass
import concourse.tile as tile
from concourse import mybir
from concourse.vector_clock import ScopedClock

BF16 = ml_dtypes.bfloat16

B, T, NZ, D = 1024, 8, 256, 256
H = 256
STEPS = 12
N_CORES = 8
PC = B // N_CORES  # 128 batch per core

F32 = mybir.dt.float32
BF = mybir.dt.bfloat16
AF = mybir.ActivationFunctionType
OP = mybir.AluOpType

# bias column layout in the (128, NBIAS) f32 bias tile
_BRZ0 = 0                      # 12 steps * 4 chunks
_BIN0 = _BRZ0 + STEPS * 4      # 12 steps * 2 chunks
_BHN0 = _BIN0 + STEPS * 2      # 2
_BRZ1 = _BHN0 + 2              # 4
_BIN1 = _BRZ1 + 4              # 2
_BHN1 = _BIN1 + 2              # 2
_BPT = _BHN1 + 2               # 2 (b_out for predT chunks)
NBIAS = _BPT + 2


def _install_tile_drain_patch():
    """walrus (CoreV3) rejects >1 sync wait on the tail drain; spill extras
    onto preceding sync nops."""
    if getattr(tile.TileContext, "_drain_patch_installed", False):
        return

    def _patched(self, tick_clock, wait_clock):
        nc = self.nc
        bb = nc.cur_bb.bb
        drain_bi = nc.sync.drain()
        drain_inst = drain_bi.ins
        wait_clock.add_sem_waits(
            drain_inst, ScopedClock({None: tick_clock.global_clock})
        )
        w = drain_inst.sync_info.on_wait if drain_inst.sync_info else None
        maxw = 1
        if w and len(w) > maxw:
            extra = list(w[maxw:])
            drain_inst.sync_info.on_wait = list(w[:maxw])
            idx = bb.instructions.index(drain_inst)
            nops = []
            for i in range(0, len(extra), maxw):
                nop_bi = nc.sync.nop()
                nop = nop_bi.ins
                si = nop.sync_info
                nop.sync_info = mybir.SyncInfo(
                    on_wait=extra[i : i + maxw],
                    on_update=(si.on_update if si else []),
                )
                bb.instructions.remove(nop)
                nops.append(nop)
            bb.instructions[idx:idx] = nops
        nc.all_engine_barrier()
        popped = nc._tile_sem_poison_stack.pop()
        assert popped is self._sem_poison
        nc.clear_and_free_semaphores(list(self.sems.allocated().values()))
        nc.all_engine_barrier()

    tile.TileContext._drain_and_barrier = _patched
    tile.TileContext._drain_patch_installed = True


def _split_waits(nc, maxw=1):
    """This walrus build rejects instructions carrying more than ~1 sem
    wait; spill extra waits onto same-engine nops placed just before."""
    for bb in nc.main_func.blocks:
        new_list = []
        for inst in bb.instructions:
            si = inst.sync_info
            w = list(si.on_wait) if si and si.on_wait else []
            if len(w) > maxw:
                keep = w[len(w) - maxw:]
                extra = w[: len(w) - maxw]
                si.on_wait = keep
                for i in range(0, len(extra), maxw):
                    nop = mybir.InstNoOp(
                        name=f"{inst.name}-sw{i}", ins=[], outs=[]
                    )
                    nop.engine = inst.engine
                    nop.sync_info = mybir.SyncInfo(
                        on_wait=extra[i : i + maxw], on_update=[]
                    )
                    nc.register_instruction(nop)
                    new_list.append(nop)
            new_list.append(inst)
        bb.instructions[:] = new_list


def build_kernel(debug_dump=()):
    """Build the per-core Bass graph (SPMD: same graph on all 8 cores)."""
    _install_tile_drain_patch()
    nc = bass.Bass()
    dbg = {}
    for tag in debug_dump:
        dbg[tag] = nc.declare_dram_parameter(f"dbg_{tag}", [128, 512], F32,
                                             isOutput=True)

    enc = nc.declare_dram_parameter("enc", [PC, NZ, D], BF, isOutput=False)
    wrz0 = nc.declare_dram_parameter("wrz0", [4, 128, 512], BF, isOutput=False)
    win0 = nc.declare_dram_parameter("win0", [2, 128, 256], BF, isOutput=False)
    whn0 = nc.declare_dram_parameter("whn0", [2, 128, 256], BF, isOutput=False)
    wrz1 = nc.declare_dram_parameter("wrz1", [4, 128, 512], BF, isOutput=False)
    win1 = nc.declare_dram_parameter("win1", [2, 128, 256], BF, isOutput=False)
    whn1 = nc.declare_dram_parameter("whn1", [2, 128, 256], BF, isOutput=False)
    wout = nc.declare_dram_parameter("wout", [2, 128, 256], BF, isOutput=False)
    biases = nc.declare_dram_parameter("biases", [128, NBIAS], F32, isOutput=False)
    boutb = nc.declare_dram_parameter("boutb", [128, 256], F32, isOutput=False)
    out = nc.declare_dram_parameter("out", [PC, STEPS, NZ], F32, isOutput=True)

    with tile.TileContext(nc) as tc:
        with (
            tc.tile_pool(name="consts", bufs=1) as consts,
            tc.tile_pool(name="state", bufs=1) as state,
            tc.tile_pool(name="enc_pool", bufs=2) as enc_pool,
            tc.tile_pool(name="gates", bufs=2) as gates,
            tc.tile_pool(name="ostage", bufs=2) as ostage,
            tc.tile_pool(name="psum", bufs=1, space="PSUM") as psum,
        ):
            # ---- weight / bias loads (overlap with enc streaming) ----
            w_sb = {}
            for name, ap, kc, mdim in (
                ("wrz0", wrz0, 4, 512),
                ("win0", win0, 2, 256),
                ("whn0", whn0, 2, 256),
                ("wrz1", wrz1, 4, 512),
                ("win1", win1, 2, 256),
                ("whn1", whn1, 2, 256),
                ("wout", wout, 2, 256),
            ):
                t_ = consts.tile([128, kc, mdim], BF, tag=name)
                nc.sync.dma_start(t_[:], ap.rearrange("k p m -> p k m"))
                w_sb[name] = t_
            bias_sb = consts.tile([128, NBIAS], F32, tag="bias")
            nc.sync.dma_start(bias_sb[:], biases[:])
            boutb_sb = consts.tile([128, 256], F32, tag="boutb")
            nc.sync.dma_start(boutb_sb[:], boutb[:])

            identity = consts.tile([128, 128], F32, tag="ident")
            nc.gpsimd.memset(identity[:], 0.0)
            nc.gpsimd.affine_select(
                out=identity[:],
                in_=identity[:],
                compare_op=OP.not_equal,
                fill=1.0,
                base=0,
                pattern=[[-1, 128]],
                channel_multiplier=1,
            )
            # prewarm the sigmoid/tanh ACT table during phase 1
            warm = consts.tile([128, 1], F32, tag="warm")
            nc.scalar.activation(warm[:], identity[:, 0:1], AF.Sigmoid)

            # ---- phase 1: context = mean over zones of enc (bf16 stream) ----
            ZCH = 64
            NCH = NZ // ZCH
            partials = state.tile([128, NCH, D], F32, tag="partials")
            for i in range(NCH):
                e_sb = enc_pool.tile([128, ZCH * D], BF, tag="echunk")
                nc.sync.dma_start(e_sb[:], enc[:, i * ZCH : (i + 1) * ZCH, :])
                # pairwise-halving tree: bf16 (2x mode) down to 8 zones,
                # then f32 tail for accuracy
                w = ZCH * D
                while w > 8 * D:
                    h = w // 2
                    nc.vector.tensor_tensor(
                        e_sb[:, 0:h], e_sb[:, 0:h], e_sb[:, h:w], OP.add
                    )
                    w = h
                tmp = gates.tile([128, 4 * D], F32, tag="redtail")
                nc.vector.tensor_tensor(
                    tmp[:], e_sb[:, 0 : 4 * D], e_sb[:, 4 * D : 8 * D], OP.add
                )
                nc.vector.tensor_tensor(
                    tmp[:, 0 : 2 * D], tmp[:, 0 : 2 * D], tmp[:, 2 * D : 4 * D],
                    OP.add,
                )
                nc.vector.tensor_tensor(
                    partials[:, i, :], tmp[:, 0:D], tmp[:, D : 2 * D], OP.add
                )
            ctx = state.tile([128, D], F32, tag="ctx")
            nc.vector.tensor_tensor(
                partials[:, 0, :], partials[:, 0, :], partials[:, 1, :], OP.add
            )
            nc.vector.tensor_tensor(
                partials[:, 2, :], partials[:, 2, :], partials[:, 3, :], OP.add
            )
            nc.vector.tensor_tensor(
                ctx[:], partials[:, 0, :], partials[:, 2, :], OP.add
            )
            nc.scalar.mul(ctx[:], ctx[:], 1.0 / NZ)

            # ---- state: actT slots ----
            actT = state.tile([128, 6, 128], BF, tag="actT")
            for c in range(2):
                ctps = psum.tile([128, 128], F32, tag="pred", bufs=1)
                nc.tensor.transpose(ctps[:], ctx[:, c * 128 : (c + 1) * 128], identity[:])
                nc.scalar.activation(actT[:, 2 + c, :], ctps[:], AF.Copy)
                nc.scalar.activation(actT[:, 4 + c, :], ctps[:], AF.Copy)
            nc.vector.memset(actT[:, 0:2, :], 0.0)

            # ---- phase 2: 12 decode steps ----
            def layer_mms_early(layer):
                """Gate matmuls that depend only on state available early:
                hn (own hidden) + rz own-hidden K-chunks. Returns psum tiles."""
                if layer == 0:
                    w_rz, w_hn = w_sb["wrz0"], w_sb["whn0"]
                    own_slots = (2, 3)        # actT slots of own hidden
                else:
                    w_rz, w_hn = w_sb["wrz1"], w_sb["whn1"]
                    own_slots = (4, 5)
                g_rz = psum.tile([128, 512], F32, tag="rz", bufs=2)
                g_hn = psum.tile([128, 256], F32, tag="hn", bufs=2)
                for m in range(2):
                    ms = slice(m * 128, (m + 1) * 128)
                    for j, slot in enumerate(own_slots):
                        nc.tensor.matmul(
                            g_hn[:, ms], w_hn[:, j, ms], actT[:, slot, :],
                            start=(m == 0 and j == 0), stop=(m == 1 and j == 1),
                        )
                for m in range(4):
                    ms = slice(m * 128, (m + 1) * 128)
                    for j, slot in enumerate(own_slots):
                        # own-hidden rows are wrz K-chunks 2,3
                        nc.tensor.matmul(
                            g_rz[:, ms], w_rz[:, 2 + j, ms], actT[:, slot, :],
                            start=(m == 0 and j == 0), stop=False,
                        )
                return g_rz, g_hn

            def layer_mms_late(layer, g_rz):
                """Gate matmuls on the fresh input (pred for L0, h0' for L1)."""
                if layer == 0:
                    w_rz, w_in = w_sb["wrz0"], w_sb["win0"]
                    inp = ((0, 0), (1, 1))
                else:
                    w_rz, w_in = w_sb["wrz1"], w_sb["win1"]
                    inp = ((0, 2), (1, 3))
                g_in = psum.tile([128, 256], F32, tag="in", bufs=2)
                for m in range(4):
                    ms = slice(m * 128, (m + 1) * 128)
                    for j, (ki, slot) in enumerate(inp):
                        nc.tensor.matmul(
                            g_rz[:, ms], w_rz[:, ki, ms], actT[:, slot, :],
                            start=False, stop=(m == 3 and j == 1),
                        )
                for m in range(2):
                    ms = slice(m * 128, (m + 1) * 128)
                    for j, (ki, slot) in enumerate(inp):
                        nc.tensor.matmul(
                            g_in[:, ms], w_in[:, ki, ms], actT[:, slot, :],
                            start=(m == 0 and j == 0), stop=(m == 1 and j == 1),
                        )
                return g_in

            def layer_chain(layer, t, g_rz, g_hn, g_in, hn_stopper):
                """Elementwise gate chain; updates actT h slots in place.
                hn_stopper: emit the g_hn stop-marking matmul is already done;
                here just consume."""
                if layer == 0:
                    h_slots = (2, 3)
                    brz_col = _BRZ0 + t * 4
                    bin_col = _BIN0 + t * 2
                    bhn_col = _BHN0
                else:
                    h_slots = (4, 5)
                    brz_col = _BRZ1
                    bin_col = _BIN1
                    bhn_col = _BHN1
                s_ = gates.tile([128, 512], BF, tag=f"S{layer}")
                for c in range(4):
                    cs = slice(c * 128, (c + 1) * 128)
                    nc.scalar.activation(
                        s_[:, cs], g_rz[:, cs], AF.Sigmoid,
                        bias=bias_sb[:, brz_col + c : brz_col + c + 1],
                    )
                t_ = gates.tile([128, 256], F32, tag=f"tt{layer}")
                v_ = gates.tile([128, 256], F32, tag=f"vv{layer}")
                for c in range(2):
                    cs = slice(c * 128, (c + 1) * 128)
                    # t = (h_n + bhn) * r
                    nc.vector.scalar_tensor_tensor(
                        t_[:, cs], g_hn[:, cs],
                        bias_sb[:, bhn_col + c : bhn_col + c + 1],
                        s_[:, cs], op0=OP.add, op1=OP.mult,
                    )
                # c = z * h  (bf16 2x, off critical path)
                c_ = gates.tile([128, 256], BF, tag=f"cc{layer}")
                hv = actT[:, h_slots[0] : h_slots[0] + 2, :].rearrange(
                    "p a b -> p (a b)"
                )
                nc.vector.tensor_tensor(c_[:], s_[:, 256:512], hv, OP.mult)
                for c in range(2):
                    cs = slice(c * 128, (c + 1) * 128)
                    # v = (i_n + bin) + t
                    nc.vector.scalar_tensor_tensor(
                        v_[:, cs], g_in[:, cs],
                        bias_sb[:, bin_col + c : bin_col + c + 1],
                        t_[:, cs], op0=OP.add, op1=OP.add,
                    )
                n_ = gates.tile([128, 256], BF, tag=f"nn{layer}")
                nc.scalar.activation(n_[:], v_[:], AF.Tanh)
                # q = (z - 1) * n ;  h' = c - q = (1-z)*n + z*h
                q_ = gates.tile([128, 256], BF, tag=f"qq{layer}")
                nc.vector.scalar_tensor_tensor(
                    q_[:], s_[:, 256:512], 1.0, n_[:],
                    op0=OP.subtract, op1=OP.mult,
                )
                nc.vector.tensor_tensor(hv, c_[:], q_[:], OP.subtract)

            for t in range(12):
                # early matmuls: depend only on h0'(t-1) / h1'(t-1)
                g_rz0, g_hn0 = layer_mms_early(0)
                g_rz1, g_hn1 = layer_mms_early(1)
                # pred-dependent matmuls (predT(t-1) written at end of t-1)
                g_in0 = layer_mms_late(0, g_rz0)
                layer_chain(0, t, g_rz0, g_hn0, g_in0, None)
                # h0'-dependent matmuls
                g_in1 = layer_mms_late(1, g_rz1)
                layer_chain(1, t, g_rz1, g_hn1, g_in1, None)
                # projection: predT (recurrence) + predB (output)
                g_pt = psum.tile([128, 256], F32, tag="pred", bufs=1)
                for m in range(2):
                    ms = slice(m * 128, (m + 1) * 128)
                    for ki, slot in ((0, 4), (1, 5)):
                        nc.tensor.matmul(
                            g_pt[:, ms], w_sb["wout"][:, ki, ms], actT[:, slot, :],
                            start=(m == 0 and ki == 0), stop=(m == 1 and ki == 1),
                        )
                g_pb = psum.tile([128, 256], F32, tag="predB", bufs=1)
                for ki, slot in ((0, 4), (1, 5)):
                    nc.tensor.matmul(
                        g_pb[:], actT[:, slot, :], w_sb["wout"][:, ki, :],
                        start=(ki == 0), stop=(ki == 1),
                    )
                for c in range(2):
                    cs = slice(c * 128, (c + 1) * 128)
                    nc.scalar.activation(
                        actT[:, c, :], g_pt[:, cs], AF.Identity,
                        bias=bias_sb[:, _BPT + c : _BPT + c + 1],
                    )
                o_ = ostage.tile([128, 256], F32, tag="ost")
                nc.vector.tensor_tensor(o_[:], g_pb[:], boutb_sb[:], OP.add)
                nc.sync.dma_start(out[:, t, :], o_[:])

    _split_waits(nc)
    return nc


def _prep_inputs(encoded_features, step_emb, W_ih0, W_hh0, b_ih0, b_hh0,
                 W_ih1, W_hh1, b_ih1, b_hh1, W_out, b_out):
    """Host-side: slice/shard the big input, transpose + cast weights,
    fold step-embedding matmul + biases into per-step bias vectors."""
    f4 = np.float32
    enc_last = np.asarray(encoded_features)[:, -1].astype(BF16)
    enc_last = np.ascontiguousarray(enc_last)

    W_ih0 = np.asarray(W_ih0, f4)
    W_hh0 = np.asarray(W_hh0, f4)
    W_ih1 = np.asarray(W_ih1, f4)
    W_hh1 = np.asarray(W_hh1, f4)
    W_out = np.asarray(W_out, f4)
    step_emb = np.asarray(step_emb, f4)
    b_ih0 = np.asarray(b_ih0, f4)
    b_hh0 = np.asarray(b_hh0, f4)
    b_ih1 = np.asarray(b_ih1, f4)
    b_hh1 = np.asarray(b_hh1, f4)
    b_out = np.asarray(b_out, f4)

    W_emb = W_ih0[:, :D]          # (768, 256)
    W_pred = W_ih0[:, D:]         # (768, 256)

    # gi_emb[t] = W_emb @ step_emb[t] + b_ih0  -> (12, 768)
    gi_emb = step_emb[:STEPS] @ W_emb.T + b_ih0[None, :]

    def kstack(*mats_cols):
        chunks = []
        for mat, cols in mats_cols:
            mt = np.ascontiguousarray(mat.T[:, cols])  # (K, M)
            for k in range(0, mt.shape[0], 128):
                chunks.append(mt[k : k + 128])
        return np.stack(chunks).astype(BF16)  # (nk, 128, M)

    rz = slice(0, 512)
    ng = slice(512, 768)
    wrz0 = kstack((W_pred, rz), (W_hh0, rz))          # K: pred0,pred1,h0c0,h0c1
    win0 = kstack((W_pred, ng))
    whn0 = kstack((W_hh0, ng))
    wrz1 = kstack((W_ih1, rz), (W_hh1, rz))           # K: h0c0,h0c1,h1c0,h1c1
    win1 = kstack((W_ih1, ng))
    whn1 = kstack((W_hh1, ng))
    wout = np.stack([np.ascontiguousarray(W_out.T)[k : k + 128] for k in (0, 128)]
                    ).astype(BF16)                    # (2, 128, 256)

    biases = np.zeros((128, NBIAS), f4)
    for t in range(STEPS):
        brz = gi_emb[t, :512] + b_hh0[:512]
        for c in range(4):
            biases[:, _BRZ0 + t * 4 + c] = brz[c * 128 : (c + 1) * 128]
        bin_ = gi_emb[t, 512:]
        for c in range(2):
            biases[:, _BIN0 + t * 2 + c] = bin_[c * 128 : (c + 1) * 128]
    for c in range(2):
        biases[:, _BHN0 + c] = b_hh0[512 + c * 128 : 512 + (c + 1) * 128]
        biases[:, _BIN1 + c] = b_ih1[512 + c * 128 : 512 + (c + 1) * 128]
        biases[:, _BHN1 + c] = b_hh1[512 + c * 128 : 512 + (c + 1) * 128]
        biases[:, _BPT + c] = b_out[c * 128 : (c + 1) * 128]
    brz1 = b_ih1[:512] + b_hh1[:512]
    for c in range(4):
        biases[:, _BRZ1 + c] = brz1[c * 128 : (c + 1) * 128]
    boutb = np.broadcast_to(b_out[None, :], (128, 256)).copy()

    shared = dict(wrz0=wrz0, win0=win0, whn0=whn0, wrz1=wrz1, win1=win1,
                  whn1=whn1, wout=wout, biases=biases, boutb=boutb)
    in_maps = []
    for i in range(N_CORES):
        m = dict(shared)
        m["enc"] = enc_last[i * PC : (i + 1) * PC]
        in_maps.append(m)
    return in_maps


_CACHE = {}


def _run(in_maps, trace=False):
    from concourse.bass_utils import run_bass_kernel_spmd

    if "nc" not in _CACHE:
        _CACHE["nc"] = build_kernel()
    nc = _CACHE["nc"]
    res = run_bass_kernel_spmd(
        nc, in_maps, core_ids=list(range(N_CORES)), trace=trace
    )
    preds = np.concatenate([res.results[i]["out"] for i in range(N_CORES)], axis=0)
    return preds, res


def kernel(encoded_features, step_emb, W_ih0, W_hh0, b_ih0, b_hh0,
           W_ih1, W_hh1, b_ih1, b_hh1, W_out, b_out, num_steps):
    assert int(num_steps) == STEPS
    in_maps = _prep_inputs(encoded_features, step_emb, W_ih0, W_hh0, b_ih0,
                           b_hh0, W_ih1, W_hh1, b_ih1, b_hh1, W_out, b_out)
    preds, _ = _run(in_maps, trace=False)
    return preds


# revision 15
# speedup vs baseline: 1.5693x; 1.0001x over previous
"""Trainium2 Bass kernel for the autoregressive GRU decode head.

Problem: context = mean over zones of encoded_features[:, -1]  -> (B, D)
then 12 autoregressive steps of a 2-layer GRU (H=256) + linear projection
to N=256 zones.  B=1024, data-parallel across 8 NeuronCores (128 batch each).

Key structure (per core, feature-major / "transposed" activations):
  actT (128p, 4 slots, 128) bf16 : [h0 c0, h0 c1, h1 c0, h1 c1]
       slot holds h[b, c*128 + p] at [p, b]    (c = chunk of the 256-dim)
  Gate tensors (PSUM/SBUF) use layout [p, c*128 + b].
  Matmuls: out(gate_chunk, B) = lhsT.T @ rhs, lhsT = W^T tile (K<=128, M=128),
  rhs = actT slot (K=128, B=128), K-chunks accumulated in PSUM.
  The prediction feedback is algebraically folded into layer 0's weights:
  W_pred @ (W_out h1 + b_out) = (W_pred W_out) h1 + W_pred b_out, so the
  recurrence never materializes pred; pred is computed batch-major only for
  the DRAM output.  Gate biases are injected as K=1 matmul rows (bias x ones)
  so the sigmoid/tanh are single wide ACT ops.
The encoded_features slice is streamed as bf16 (host-converted); the zone
mean is a pairwise TT-add tree on DVE (bf16 2x levels, f32 tail).
"""

import sys

for _p in ("/opt/trn_rl_repo",):
    if _p not in sys.path:
        sys.path.insert(0, _p)

import numpy as np
import ml_dtypes

import concourse.bass as bass
import concourse.tile as tile
from concourse import mybir
from concourse.vector_clock import ScopedClock

BF16 = ml_dtypes.bfloat16

B, T, NZ, D = 1024, 8, 256, 256
H = 256
STEPS = 12
N_CORES = 8
PC = B // N_CORES  # 128 batch per core

F32 = mybir.dt.float32
BF = mybir.dt.bfloat16
AF = mybir.ActivationFunctionType
OP = mybir.AluOpType

# bias-row column layout (each slot is 128 wide) in the (1, NBROW*128) bf16
# bias-rows tensor: value[slot*128 + j] is the bias for gate index
# (chunk c of the group) at position j
_RZ0 = 0                      # 12 steps * 4 chunks
_IN0 = _RZ0 + STEPS * 4       # 12 steps * 2 chunks
_HN0 = _IN0 + STEPS * 2       # 2
_RZ1 = _HN0 + 2               # 4
_IN1 = _RZ1 + 4               # 2
_HN1 = _IN1 + 2               # 2
NBROW = _HN1 + 2


def _install_tile_drain_patch():
    """walrus (CoreV3) rejects >1 sync wait on the tail drain; spill extras
    onto preceding sync nops."""
    if getattr(tile.TileContext, "_drain_patch_installed", False):
        return

    def _patched(self, tick_clock, wait_clock):
        nc = self.nc
        bb = nc.cur_bb.bb
        drain_bi = nc.sync.drain()
        drain_inst = drain_bi.ins
        wait_clock.add_sem_waits(
            drain_inst, ScopedClock({None: tick_clock.global_clock})
        )
        w = drain_inst.sync_info.on_wait if drain_inst.sync_info else None
        maxw = 1
        if w and len(w) > maxw:
            extra = list(w[maxw:])
            drain_inst.sync_info.on_wait = list(w[:maxw])
            idx = bb.instructions.index(drain_inst)
            nops = []
            for i in range(0, len(extra), maxw):
                nop_bi = nc.sync.nop()
                nop = nop_bi.ins
                si = nop.sync_info
                nop.sync_info = mybir.SyncInfo(
                    on_wait=extra[i : i + maxw],
                    on_update=(si.on_update if si else []),
                )
                bb.instructions.remove(nop)
                nops.append(nop)
            bb.instructions[idx:idx] = nops
        nc.all_engine_barrier()
        popped = nc._tile_sem_poison_stack.pop()
        assert popped is self._sem_poison
        nc.clear_and_free_semaphores(list(self.sems.allocated().values()))
        nc.all_engine_barrier()

    tile.TileContext._drain_and_barrier = _patched
    tile.TileContext._drain_patch_installed = True


def _split_waits(nc, maxw=1):
    """This walrus build rejects instructions carrying more than ~1 sem
    wait; spill extra waits onto same-engine nops placed just before."""
    for bb in nc.main_func.blocks:
        new_list = []
        for inst in bb.instructions:
            si = inst.sync_info
            w = list(si.on_wait) if si and si.on_wait else []
            if len(w) > maxw:
                keep = w[len(w) - maxw:]
                extra = w[: len(w) - maxw]
                si.on_wait = keep
                for i in range(0, len(extra), maxw):
                    nop = mybir.InstNoOp(
                        name=f"{inst.name}-sw{i}", ins=[], outs=[]
                    )
                    nop.engine = inst.engine
                    nop.sync_info = mybir.SyncInfo(
                        on_wait=extra[i : i + maxw], on_update=[]
                    )
                    nc.register_instruction(nop)
                    new_list.append(nop)
            new_list.append(inst)
        bb.instructions[:] = new_list


class _Group:
    """Tracks start/stop flags for a PSUM accumulation group whose matmuls
    are emitted in several program-order batches."""

    def __init__(self, total):
        self.total = total
        self.emitted = 0

    def flags(self):
        start = self.emitted == 0
        self.emitted += 1
        return start, self.emitted == self.total


def build_kernel(nsteps=12):
    """Build the per-core Bass graph (SPMD: same graph on all 8 cores)."""
    _install_tile_drain_patch()
    nc = bass.Bass()

    enc = nc.declare_dram_parameter("enc", [PC, NZ, D], BF, isOutput=False)
    wrz0 = nc.declare_dram_parameter("wrz0", [4, 128, 512], BF, isOutput=False)
    win0 = nc.declare_dram_parameter("win0", [2, 128, 256], BF, isOutput=False)
    whn0 = nc.declare_dram_parameter("whn0", [2, 128, 256], BF, isOutput=False)
    wrz1 = nc.declare_dram_parameter("wrz1", [4, 128, 512], BF, isOutput=False)
    win1 = nc.declare_dram_parameter("win1", [2, 128, 256], BF, isOutput=False)
    whn1 = nc.declare_dram_parameter("whn1", [2, 128, 256], BF, isOutput=False)
    wout = nc.declare_dram_parameter("wout", [2, 128, 256], BF, isOutput=False)
    brows = nc.declare_dram_parameter("brows", [1, NBROW * 128], BF, isOutput=False)
    boutb = nc.declare_dram_parameter("boutb", [128, 256], F32, isOutput=False)
    out = nc.declare_dram_parameter("out", [PC, STEPS, NZ], F32, isOutput=True)

    with tile.TileContext(nc) as tc:
        with (
            tc.tile_pool(name="consts", bufs=1) as consts,
            tc.tile_pool(name="state", bufs=1) as state,
            tc.tile_pool(name="enc_pool", bufs=2) as enc_pool,
            tc.tile_pool(name="gates", bufs=2) as gates,
            tc.tile_pool(name="ostage", bufs=2) as ostage,
            tc.tile_pool(name="psum", bufs=1, space="PSUM") as psum,
        ):
            # ---- phase 1 DMA: encoded chunks first on the sync queue ----
            ZCH = 32
            NCH = NZ // ZCH
            e_tiles = []
            for i in range(NCH):
                e_sb = enc_pool.tile([128, ZCH * D], BF, tag="echunk")
                nc.sync.dma_start(e_sb[:], enc[:, i * ZCH : (i + 1) * ZCH, :])
                e_tiles.append(e_sb)

            # ---- weight / bias loads on the scalar HWDGE queue ----
            w_sb = {}
            for name, ap, kc, mdim in (
                ("wrz0", wrz0, 4, 512),
                ("win0", win0, 2, 256),
                ("whn0", whn0, 2, 256),
                ("wrz1", wrz1, 4, 512),
                ("win1", win1, 2, 256),
                ("whn1", whn1, 2, 256),
                ("wout", wout, 2, 256),
            ):
                t_ = consts.tile([128, kc, mdim], BF, tag=name)
                nc.scalar.dma_start(t_[:], ap.rearrange("k p m -> p k m"))
                w_sb[name] = t_
            brow_sb = consts.tile([1, NBROW * 128], BF, tag="brow")
            nc.scalar.dma_start(brow_sb[:], brows[:])
            boutb_sb = consts.tile([128, 256], F32, tag="boutb")
            nc.scalar.dma_start(boutb_sb[:], boutb[:])

            ones_row = consts.tile([1, 128], BF, tag="ones")
            nc.gpsimd.memset(ones_row[:], 1.0)
            identity = consts.tile([128, 128], F32, tag="ident")
            nc.gpsimd.memset(identity[:], 0.0)
            nc.gpsimd.affine_select(
                out=identity[:],
                in_=identity[:],
                compare_op=OP.not_equal,
                fill=1.0,
                base=0,
                pattern=[[-1, 128]],
                channel_multiplier=1,
            )
            # prewarm the sigmoid/tanh ACT table during phase 1
            warm = consts.tile([128, 1], F32, tag="warm")
            nc.scalar.activation(warm[:], identity[:, 0:1], AF.Sigmoid)

            # ---- phase 1: zone-mean tree per chunk on DVE ----
            partials = state.tile([128, NCH, D], F32, tag="partials")
            for i in range(NCH):
                e_sb = e_tiles[i]
                w = ZCH * D
                while w > 8 * D:
                    h = w // 2
                    nc.vector.tensor_tensor(
                        e_sb[:, 0:h], e_sb[:, 0:h], e_sb[:, h:w], OP.add
                    )
                    w = h
                tmp = gates.tile([128, 4 * D], F32, tag="redtail")
                nc.vector.tensor_tensor(
                    tmp[:], e_sb[:, 0 : 4 * D], e_sb[:, 4 * D : 8 * D], OP.add
                )
                nc.vector.tensor_tensor(
                    tmp[:, 0 : 2 * D], tmp[:, 0 : 2 * D], tmp[:, 2 * D : 4 * D],
                    OP.add,
                )
                nc.vector.tensor_tensor(
                    partials[:, i, :], tmp[:, 0:D], tmp[:, D : 2 * D], OP.add
                )
            ctx = state.tile([128, D], F32, tag="ctx")
            stride = 1
            while stride < NCH:
                for i in range(0, NCH, 2 * stride):
                    nc.vector.tensor_tensor(
                        partials[:, i, :], partials[:, i, :],
                        partials[:, i + stride, :], OP.add,
                    )
                stride *= 2
            nc.scalar.mul(ctx[:], partials[:, 0, :], 1.0 / NZ)

            # ---- state: actT slots [h0c0, h0c1, h1c0, h1c1] ----
            actT = state.tile([128, 4, 128], BF, tag="actT")
            for c in range(2):
                ctps = psum.tile([128, 128], F32, tag="pred", bufs=1)
                nc.tensor.transpose(
                    ctps[:], ctx[:, c * 128 : (c + 1) * 128], identity[:]
                )
                nc.scalar.activation(actT[:, c, :], ctps[:], AF.Copy)
                nc.scalar.activation(actT[:, 2 + c, :], ctps[:], AF.Copy)

            def brow(slot):
                return brow_sb[0:1, slot * 128 : (slot + 1) * 128]

            def bias_mms(g, grp, base, nch):
                for c in range(nch):
                    st, sp = grp.flags()
                    nc.tensor.matmul(
                        g[:, c * 128 : (c + 1) * 128], brow(base + c),
                        ones_row[:], start=st, stop=sp,
                    )

            def gate_mms(g, grp, w_t, kis, slots, nch):
                for m in range(nch):
                    ms = slice(m * 128, (m + 1) * 128)
                    for ki, slot in zip(kis, slots):
                        st, sp = grp.flags()
                        nc.tensor.matmul(
                            g[:, ms], w_t[:, ki, ms], actT[:, slot, :],
                            start=st, stop=sp,
                        )

            def chain(layer, g_rz, g_hn, g_in):
                h_sl = (0, 1) if layer == 0 else (2, 3)
                s_ = gates.tile([128, 512], BF, tag=f"S{layer}")
                nc.scalar.activation(s_[:], g_rz[:], AF.Sigmoid)
                t_ = gates.tile([128, 256], F32, tag=f"tt{layer}")
                v_ = gates.tile([128, 256], F32, tag=f"vv{layer}")
                nc.vector.tensor_tensor(t_[:], g_hn[:], s_[:, 0:256], OP.mult)
                nc.vector.tensor_tensor(v_[:], g_in[:], t_[:], OP.add)
                c_ = gates.tile([128, 256], BF, tag=f"cc{layer}")
                hv = actT[:, h_sl[0] : h_sl[0] + 2, :].rearrange("p a b -> p (a b)")
                nc.vector.tensor_tensor(c_[:], s_[:, 256:512], hv, OP.mult)
                n_ = gates.tile([128, 256], BF, tag=f"nn{layer}")
                nc.scalar.activation(n_[:], v_[:], AF.Tanh)
                q_ = gates.tile([128, 256], BF, tag=f"qq{layer}")
                nc.vector.scalar_tensor_tensor(
                    q_[:], s_[:, 256:512], 1.0, n_[:],
                    op0=OP.subtract, op1=OP.mult,
                )
                nc.vector.tensor_tensor(hv, c_[:], q_[:], OP.subtract)

            # ---- phase 2: 12 decode steps ----
            for t in range(nsteps):
                inp0 = t > 0  # step 0 has zero pred feedback
                g_rz0 = psum.tile([128, 512], F32, tag="rz", bufs=2)
                g_hn0 = psum.tile([128, 256], F32, tag="hn", bufs=2)
                g_in0 = psum.tile([128, 256], F32, tag="in", bufs=2)
                g_rz1 = psum.tile([128, 512], F32, tag="rz", bufs=2)
                g_hn1 = psum.tile([128, 256], F32, tag="hn", bufs=2)
                g_in1 = psum.tile([128, 256], F32, tag="in", bufs=2)
                grz0 = _Group(4 + 8 + (8 if inp0 else 0))
                ghn0 = _Group(2 + 4)
                gin0 = _Group(2 + (4 if inp0 else 0))
                grz1 = _Group(4 + 8 + 8)
                ghn1 = _Group(2 + 4)
                gin1 = _Group(2 + 4)

                # everything for layer 0 is available at step start
                bias_mms(g_rz0, grz0, _RZ0 + t * 4, 4)
                bias_mms(g_in0, gin0, _IN0 + t * 2, 2)
                if inp0:
                    # pred feedback folded onto h1(t-1): wrz0/win0 K-chunks 0,1
                    gate_mms(g_rz0, grz0, w_sb["wrz0"], (0, 1), (2, 3), 4)
                    gate_mms(g_in0, gin0, w_sb["win0"], (0, 1), (2, 3), 2)
                bias_mms(g_hn0, ghn0, _HN0, 2)
                gate_mms(g_rz0, grz0, w_sb["wrz0"], (2, 3), (0, 1), 4)
                gate_mms(g_hn0, ghn0, w_sb["whn0"], (0, 1), (0, 1), 2)
                # layer 1 own-hidden part (needs h1(t-1) only)
                bias_mms(g_rz1, grz1, _RZ1, 4)
                bias_mms(g_hn1, ghn1, _HN1, 2)
                gate_mms(g_rz1, grz1, w_sb["wrz1"], (2, 3), (2, 3), 4)
                gate_mms(g_hn1, ghn1, w_sb["whn1"], (0, 1), (2, 3), 2)

                chain(0, g_rz0, g_hn0, g_in0)

                # layer 1 input part (needs h0')
                bias_mms(g_in1, gin1, _IN1, 2)
                gate_mms(g_rz1, grz1, w_sb["wrz1"], (0, 1), (0, 1), 4)
                gate_mms(g_in1, gin1, w_sb["win1"], (0, 1), (0, 1), 2)

                chain(1, g_rz1, g_hn1, g_in1)

                # output projection, batch-major (off the recurrence)
                g_pb = psum.tile([128, 256], F32, tag="predB", bufs=1)
                for ki, slot in ((0, 2), (1, 3)):
                    nc.tensor.matmul(
                        g_pb[:], actT[:, slot, :], w_sb["wout"][:, ki, :],
                        start=(ki == 0), stop=(ki == 1),
                    )
                o_ = ostage.tile([128, 256], F32, tag="ost")
                nc.vector.tensor_tensor(o_[:], g_pb[:], boutb_sb[:], OP.add)
                nc.sync.dma_start(out[:, t, :], o_[:])

    _split_waits(nc)
    return nc


def _prep_inputs(encoded_features, step_emb, W_ih0, W_hh0, b_ih0, b_hh0,
                 W_ih1, W_hh1, b_ih1, b_hh1, W_out, b_out):
    """Host-side: slice/shard the big input, transpose + cast weights,
    fold the output projection into layer-0 input weights, fold the
    step-embedding matmul + all additive constants into bias rows."""
    f4 = np.float32
    enc_last = np.asarray(encoded_features)[:, -1].astype(BF16)
    enc_last = np.ascontiguousarray(enc_last)

    W_ih0 = np.asarray(W_ih0, f4)
    W_hh0 = np.asarray(W_hh0, f4)
    W_ih1 = np.asarray(W_ih1, f4)
    W_hh1 = np.asarray(W_hh1, f4)
    W_out = np.asarray(W_out, f4)
    step_emb = np.asarray(step_emb, f4)
    b_ih0 = np.asarray(b_ih0, f4)
    b_hh0 = np.asarray(b_hh0, f4)
    b_ih1 = np.asarray(b_ih1, f4)
    b_hh1 = np.asarray(b_hh1, f4)
    b_out = np.asarray(b_out, f4)

    W_emb = W_ih0[:, :D]          # (768, 256)
    W_pred = W_ih0[:, D:]         # (768, 256)
    W_fold = W_pred @ W_out       # (768, 256): pred feedback folded onto h1
    b_fold = W_pred @ b_out       # (768,)

    # gi_emb[t] = W_emb @ step_emb[t] + b_ih0  -> (12, 768)
    gi_emb = step_emb[:STEPS] @ W_emb.T + b_ih0[None, :]

    def kstack(*mats_cols):
        chunks = []
        for mat, cols in mats_cols:
            mt = np.ascontiguousarray(mat.T[:, cols])  # (K, M)
            for k in range(0, mt.shape[0], 128):
                chunks.append(mt[k : k + 128])
        return np.stack(chunks).astype(BF16)  # (nk, 128, M)

    rz = slice(0, 512)
    ng = slice(512, 768)
    wrz0 = kstack((W_fold, rz), (W_hh0, rz))          # K: h1c0,h1c1,h0c0,h0c1
    win0 = kstack((W_fold, ng))
    whn0 = kstack((W_hh0, ng))
    wrz1 = kstack((W_ih1, rz), (W_hh1, rz))           # K: h0c0,h0c1,h1c0,h1c1
    win1 = kstack((W_ih1, ng))
    whn1 = kstack((W_hh1, ng))
    wout = np.stack([np.ascontiguousarray(W_out.T)[k : k + 128] for k in (0, 128)]
                    ).astype(BF16)                    # (2, 128, 256)

    brows = np.zeros(NBROW * 128, f4)

    def put(base, vec):
        brows[base * 128 : base * 128 + len(vec)] = vec

    for t in range(STEPS):
        extra = b_fold if t > 0 else 0.0
        put(_RZ0 + t * 4, gi_emb[t, :512] + b_hh0[:512] + (extra[:512] if t else 0.0))
        put(_IN0 + t * 2, gi_emb[t, 512:] + (extra[512:] if t else 0.0))
    put(_HN0, b_hh0[512:])
    put(_RZ1, b_ih1[:512] + b_hh1[:512])
    put(_IN1, b_ih1[512:])
    put(_HN1, b_hh1[512:])
    brows = brows.astype(BF16)[None, :]
    boutb = np.broadcast_to(b_out[None, :], (128, 256)).astype(f4).copy()

    shared = dict(wrz0=wrz0, win0=win0, whn0=whn0, wrz1=wrz1, win1=win1,
                  whn1=whn1, wout=wout, brows=brows, boutb=boutb)
    in_maps = []
    for i in range(N_CORES):
        m = dict(shared)
        m["enc"] = enc_last[i * PC : (i + 1) * PC]
        in_maps.append(m)
    return in_maps


_CACHE = {}


def _run(in_maps, trace=False):
    from concourse.bass_utils import run_bass_kernel_spmd

    if "nc" not in _CACHE:
        _CACHE["nc"] = build_kernel()
    nc = _CACHE["nc"]
    res = run_bass_kernel_spmd(
        nc, in_maps, core_ids=list(range(N_CORES)), trace=trace
    )
    preds = np.concatenate([res.results[i]["out"] for i in range(N_CORES)], axis=0)
    return preds, res


def kernel(encoded_features, step_emb, W_ih0, W_hh0, b_ih0, b_hh0,
           W_ih1, W_hh1, b_ih1, b_hh1, W_out, b_out, num_steps):
    assert int(num_steps) == STEPS
    in_maps = _prep_inputs(encoded_features, step_emb, W_ih0, W_hh0, b_ih0,
                           b_hh0, W_ih1, W_hh1, b_ih1, b_hh1, W_out, b_out)
    preds, _ = _run(in_maps, trace=False)
    return preds
